# revision 10
# baseline (speedup 1.0000x reference)
"""ChebyshevKANLayer Trainium2 kernel (8 NeuronCores, data-parallel over batch).

Math:
    y[b,o] = sum_{i,j} U_j(tanh(x[b,i])) * C[i,o,j],  U_j = Chebyshev 2nd kind

Device strategy (v2 — "stationary-streams"):
    8 bounded basis streams generated on-chip from u = tanh(x):
        u, q2=u^2, p3=u*q2, q4=T2^2 (T2=2q2-1), p5=u*q4, p6=q2*q4,
        p7=p3*q4, p8=q4*q4
    (distinct leading degrees 1..8 -> independent; all |f|<=1 on [-1,1];
    change-of-basis coeffs <= 32 so fp16 stream rounding stays small).

    PE schedule: the STREAM tile is the stationary operand [128 i, 128 b]
    and the tiny weight tile W_{k,ib} [128 i, 32 o] is the moving operand,
    so every matmul runs the full 128x128 array (M=128) and streams only
    N=32 rows. 32 accumulating matmuls per 128-batch block produce
    PSUM [128 b, 32 o] directly in output layout. Measured on HW:
    35.4 ns/matmul (vs 238 ns for the M=32/N=512 orientation) ->
    ~72us PE time/core, the design bottleneck.

    Engine balance per 1024-col chunk: ACT: 4x tanh + 4x Square(2q2-1)
    + PSUM->SBUF cast-copy; DVE: 14 muls; Pool: 10 muls. Output written
    fp16 [128 b-part, 64 blk * 32 o] (0.5 MB/core), folded on host.
"""

import os
import sys

import numpy as np

for _p in ("/opt/trn_rl_repo", "/root/.axon_site/_ro/trn_rl_repo"):
    if os.path.isdir(_p) and _p not in sys.path:
        sys.path.insert(0, _p)

import concourse.bacc as bacc  # noqa: E402
import concourse.bass as bass  # noqa: E402
import concourse.tile as tile  # noqa: E402
from concourse import mybir  # noqa: E402
from concourse.bass_utils import run_bass_kernel_spmd  # noqa: E402

B, I, O, DEG = 65536, 512, 32, 8
N_CORES = 8
B_SHARD = B // N_CORES  # 8192
F = 1024                # chunk width (b columns)
N_IB = 4                # 128-row i-blocks
N_K = 8                 # streamed basis functions
N_BLK = F // 128        # 128-b matmul blocks per chunk
F16 = mybir.dt.float16
F32 = mybir.dt.float32

_LAST_RESULTS = None  # test.py reads exec_time_ns from here


def _pmul(a, b):
    return np.convolve(a, b)


def _pad(a, n):
    out = np.zeros(n)
    out[: len(a)] = a
    return out


def _host_weights(cheby_coeffs):
    """Change of basis U_j -> {1, u, q2, p3, q4, p5, p6, p7, p8}."""
    X = np.array([0.0, 1.0])
    one = np.array([1.0])
    u = X
    q2 = _pmul(u, u)
    t2 = 2 * q2 - _pad(one, len(q2))
    q4 = _pmul(t2, t2)
    p3 = _pmul(u, q2)
    p5 = _pmul(u, q4)
    p6 = _pmul(q2, q4)
    p7 = _pmul(p3, q4)
    p8 = _pmul(q4, q4)
    basis = [u, q2, p3, q4, p5, p6, p7, p8]  # stream order (k index)

    Bm = np.zeros((9, 9))
    Bm[0, 0] = 1.0
    for k, p in enumerate(basis):
        Bm[: len(p), 1 + k] = p
    U = [np.array([1.0]), 2 * X]
    for n in range(2, DEG + 1):
        U.append(_pad(2 * _pmul(X, U[n - 1]), n + 1) - _pad(U[n - 2], n + 1))
    Um = np.zeros((9, 9))
    for j, p in enumerate(U):
        Um[: len(p), j] = p
    beta = np.linalg.solve(Bm, Um)  # [basis(1+8), j(9)]

    C = cheby_coeffs.astype(np.float64)  # [I, O, 9]
    Wk = np.einsum("ioj,kj->kio", C, beta[1:])  # [8, I, O]
    bias = np.einsum("ioj,j->o", C, beta[0])  # [O]

    # Device layout: [128 part(i sub), 32 tiles * 32 o]; tile t = k*N_IB+ib
    Whost = np.zeros((128, N_K * N_IB * O), np.float16)
    for k in range(N_K):
        for ib in range(N_IB):
            t = k * N_IB + ib
            Whost[:, t * O : (t + 1) * O] = Wk[k, 128 * ib : 128 * (ib + 1), :]
    return Whost, bias.astype(np.float32)


_NC_CACHE = {}


def _build_bass(repeat=1):
    if repeat in _NC_CACHE:
        return _NC_CACHE[repeat]
    nc = bacc.Bacc()
    xT = nc.dram_tensor("xT", [I, B_SHARD], F16, kind="ExternalInput")
    Wd = nc.dram_tensor("W", [128, N_K * N_IB * O], F16, kind="ExternalInput")
    # y in [b mod 128 (partitions), blk*32+o] layout, fp16 (ACT cast-copy
    # from PSUM, deferred one chunk so ACT never stalls on PE's tail)
    yD = nc.dram_tensor("yD", [128, (B_SHARD // 128) * O], F16, kind="ExternalOutput")

    Tanh = mybir.ActivationFunctionType.Tanh
    Square = mybir.ActivationFunctionType.Square

    n_chunks = B_SHARD // F

    with tile.TileContext(nc) as tc:
        with (
            tc.tile_pool(name="consts", bufs=1) as consts,
            tc.tile_pool(name="xs", bufs=2) as xs,
            tc.tile_pool(name="strm", bufs=2) as strm,
            tc.tile_pool(name="outp", bufs=2) as outp,
            tc.tile_pool(name="psum", bufs=2, space="PSUM") as psum,
        ):
            w_sb = consts.tile([128, N_K * N_IB * O], F16)
            nc.sync.dma_start(out=w_sb, in_=Wd[:, :])
            neg1 = consts.tile([128, 1], F32)
            nc.vector.memset(neg1, -1.0)

            def flush(c, ps):
                ycp = outp.tile([128, N_BLK * O], F16, tag="ycp")
                nc.scalar.copy(ycp, ps)  # ACT: cast fp32 -> fp16
                nc.sync.dma_start(
                    out=yD[:, c * (N_BLK * O) : (c + 1) * (N_BLK * O)], in_=ycp
                )

            def chunk_body(c):
                streams = {}  # (k, ib) -> tile [128, F]
                for ib in range(N_IB):
                    xt = xs.tile([128, F], F16, tag=f"x{ib}")
                    nc.sync.dma_start(
                        out=xt,
                        in_=xT[128 * ib : 128 * (ib + 1), c * F : (c + 1) * F],
                    )

                    def st(name):
                        return strm.tile(
                            [128, F], F16, tag=f"{name}{ib}", name=f"{name}{ib}"
                        )

                    # engine split: DVE 14 muls, Pool 10 muls per chunk
                    dve_mul = nc.vector.tensor_mul
                    pool_mul = nc.gpsimd.tensor_mul
                    if ib < 2:
                        m_q2, m_p3, m_p5, m_p6 = (dve_mul,) * 4
                        m_p7, m_p8 = (pool_mul,) * 2
                    else:
                        m_q2, m_p3, m_p5 = (dve_mul,) * 3
                        m_p6, m_p7, m_p8 = (pool_mul,) * 3

                    u = st("u")
                    nc.scalar.activation(u, xt, Tanh)
                    q2 = st("q2")
                    m_q2(q2, u, u)
                    q4 = st("q4")  # (2*q2-1)^2 = T2^2
                    nc.scalar.activation(q4, q2, Square, bias=neg1, scale=2.0)
                    p3 = st("p3")
                    m_p3(p3, u, q2)
                    p5 = st("p5")
                    m_p5(p5, u, q4)
                    p6 = st("p6")
                    m_p6(p6, q2, q4)
                    p7 = st("p7")
                    m_p7(p7, p3, q4)
                    p8 = st("p8")
                    m_p8(p8, q4, q4)
                    for k, s in enumerate([u, q2, p3, q4, p5, p6, p7, p8]):
                        streams[(k, ib)] = s

                ps = psum.tile([128, N_BLK * O], F32, tag="ps")
                for blk in range(N_BLK):
                    t = 0
                    for k in range(N_K):
                        for ib in range(N_IB):
                            nc.tensor.matmul(
                                ps[:, blk * O : (blk + 1) * O],
                                lhsT=streams[(k, ib)][
                                    :, blk * 128 : (blk + 1) * 128
                                ],
                                rhs=w_sb[:, (k * N_IB + ib) * O : (k * N_IB + ib + 1) * O],
                                start=(t == 0),
                                stop=(t == N_K * N_IB - 1),
                                skip_group_check=True,
                            )
                            t += 1
                return ps

            def run_pass():
                pending = None  # (chunk idx, psum tile) awaiting flush
                for c in range(n_chunks):
                    ps = chunk_body(c)
                    if pending is not None:
                        flush(*pending)
                    pending = (c, ps)
                flush(*pending)

            if repeat == 1:
                run_pass()
            else:
                with tc.For_i(0, repeat):
                    run_pass()

    nc.compile()
    _NC_CACHE[repeat] = nc
    return nc


def _unshard(yD_core):
    """yD [128, 64*32] -> y [B_SHARD, 32] float32."""
    n_blk_total = B_SHARD // 128
    y = yD_core.astype(np.float32).reshape(128, n_blk_total, O)
    return y.transpose(1, 0, 2).reshape(B_SHARD, O)


def kernel(x, cheby_coeffs):
    global _LAST_RESULTS
    assert x.shape == (B, I) and cheby_coeffs.shape == (I, O, DEG + 1)
    Whost, bias = _host_weights(cheby_coeffs)
    xT16 = np.ascontiguousarray(x.T.astype(np.float16))  # [I, B]

    nc = _build_bass()
    in_maps = []
    for c in range(N_CORES):
        shard = np.ascontiguousarray(xT16[:, c * B_SHARD : (c + 1) * B_SHARD])
        in_maps.append({"xT": shard, "W": Whost})
    res = run_bass_kernel_spmd(nc, in_maps, core_ids=list(range(N_CORES)))
    _LAST_RESULTS = res
    parts = [_unshard(r["yD"]) for r in res.results]
    y = np.concatenate(parts, axis=0)  # [B, O]
    return (y + bias[None, :]).astype(np.float32)
